# revision 1
# baseline (speedup 1.0000x reference)
"""Trainium2 Bass kernel for nn_ConceptLayer (B=8, S=2048, E=128).

out[b,s,c] = LN( einsum('sa,sp,cap->sc', h[b], s_seq[b], W) + h[b] )
  h = x @ dense_w + dense_b
  s_seq = decayed prefix sum of h along s (s_i = (s_{i-1}+h_{i-1})/1.2)

Sharding: data-parallel over batch, one sample per NeuronCore (8 cores).
concept_map replicated (host-pretransposed to [p, (a,c)] bf16); x is
host-pretransposed to xT [e, t] so no on-device transposes are needed.

Per-core pipeline:
  1. hT-chunk matmuls (dense_w.T @ xT, fp32); PSUM->SBUF copy applies
     (+bias)*1/DECAY producing the scan feed hTp (shifted one col)
  2. s_seq^T via tensor_tensor_scan (state = state/d + (h+b)/d) in fp32,
     downcast to bf16 (sTmm) for the tensor-engine operand
  3. h natural layout per 128-token block: matmul(lhsT=xT slice, rhs=dense_w)
     plus a K=1 ones-x-bias matmul into the same PSUM
  4. per block: acc = h_blk (residual); 32 matmuls N=512
     (lhsT = sT block stationary, rhs = W2 slices) -> Y in PSUM;
     fused axpy acc += Y[:,a-slice] * h[:,a] (scalar_tensor_tensor)
  5. LayerNorm (bn_stats/bn_aggr, sqrt, reciprocal) + gamma/beta, DMA out
"""

import os
import sys

import numpy as np

for _p in ("/opt/trn_rl_repo",):
    if _p not in sys.path and os.path.isdir(_p):
        sys.path.insert(0, _p)

import concourse.bass as bass
import concourse.bacc as bacc
import concourse.tile as tile
from concourse import mybir
from concourse.bass_utils import run_bass_kernel_spmd

B, S, E = 8, 2048, 128
DECAY = 1.2
LN_EPS = 1e-3
NBLK = S // 128          # 16 token blocks per core
NCHUNK = (E * E) // 512  # 32 matmul chunks of 512 (a,c) columns per block
F32 = mybir.dt.float32
BF16 = mybir.dt.bfloat16
NPBF16 = mybir.dt.np(BF16)

_CACHE = {}
LAST_RESULT = None  # BassKernelResults of the most recent run (for test.py)


def _build_nc():
    nc = bacc.Bacc(None, target_bir_lowering=False)

    xT_d = nc.declare_dram_parameter("xT", [E, S], BF16, isOutput=False)
    dw_d = nc.declare_dram_parameter("dense_w", [E, E], BF16, isOutput=False)
    bc_d = nc.declare_dram_parameter("b_col", [E, 1], F32, isOutput=False)
    br_d = nc.declare_dram_parameter("b_row", [1, E], BF16, isOutput=False)
    w2_d = nc.declare_dram_parameter("w2", [E, E * E], BF16, isOutput=False)
    gam_d = nc.declare_dram_parameter("gamma_rep", [128, E], F32, isOutput=False)
    bet_d = nc.declare_dram_parameter("beta_rep", [128, E], F32, isOutput=False)
    out_d = nc.declare_dram_parameter("out", [S, E], F32, isOutput=True)

    with tile.TileContext(nc) as tc:
        with (
            tc.tile_pool(name="singles", bufs=1) as singles,
            tc.tile_pool(name="blk", bufs=2) as blk,
            tc.tile_pool(name="small", bufs=4) as small,
            tc.tile_pool(name="h_ps", bufs=2, space="PSUM") as h_ps,
            tc.tile_pool(name="hn_ps", bufs=2, space="PSUM") as hn_ps,
            tc.tile_pool(name="y_ps", bufs=4, space="PSUM") as y_ps,
        ):
            # ---- resident tensors ----
            w2_sb = singles.tile([E, E * E], BF16)
            nc.sync.dma_start(out=w2_sb[:], in_=w2_d[:])
            dw_sb = singles.tile([E, E], BF16)
            nc.sync.dma_start(out=dw_sb[:], in_=dw_d[:])
            bcol = singles.tile([E, 1], F32)
            nc.sync.dma_start(out=bcol[:], in_=bc_d[:])
            brow = singles.tile([1, E], BF16)
            nc.sync.dma_start(out=brow[:], in_=br_d[:])
            gam_sb = singles.tile([128, E], F32)
            nc.sync.dma_start(out=gam_sb[:], in_=gam_d[:])
            bet_sb = singles.tile([128, E], F32)
            nc.sync.dma_start(out=bet_sb[:], in_=bet_d[:])
            xT = singles.tile([E, S], BF16)
            nc.sync.dma_start(out=xT[:], in_=xT_d[:])

            eps_t = singles.tile([128, 1], F32)
            nc.vector.memset(eps_t[:], LN_EPS)
            dinv = singles.tile([128, 512], F32)
            nc.vector.memset(dinv[:], 1.0 / DECAY)
            ones1 = singles.tile([1, 128], BF16)
            nc.vector.memset(ones1[:], 1.0)

            hTp = singles.tile([E, S + 1], F32)   # col j+1 = (h_j+b)/d, col0 = 0
            sT32 = singles.tile([E, S], F32)
            sTmm = singles.tile([E, S], BF16)
            h_sb = singles.tile([128, NBLK, E], F32)  # h natural, blocked

            nc.vector.memset(hTp[:, 0:1], 0.0)

            # ---- 1. hT chunks = dense_w.T @ xT; scan feed with (+b)/d ----
            for q in range(4):
                hp = h_ps.tile([E, 512], F32)
                nc.tensor.matmul(hp[:], dw_sb[:], xT[:, q * 512:(q + 1) * 512],
                                 start=True, stop=True)
                nc.vector.tensor_scalar(
                    hTp[:, q * 512 + 1:(q + 1) * 512 + 1], hp[:], bcol[:],
                    1.0 / DECAY, mybir.AluOpType.add, mybir.AluOpType.mult)

            # ---- 2. decay prefix scan -> sT ----
            for q in range(4):
                lo, hi = q * 512, (q + 1) * 512
                init = 0.0 if q == 0 else sT32[:, lo - 1:lo]
                nc.vector.tensor_tensor_scan(
                    sT32[:, lo:hi], dinv[:], hTp[:, lo:hi], init,
                    mybir.AluOpType.mult, mybir.AluOpType.add)
                nc.gpsimd.tensor_copy(out=sTmm[:, lo:hi], in_=sT32[:, lo:hi])

            # ---- 3. h natural layout: h_blk = xT_blk.T @ dense_w + 1s*b ----
            for g in range(NBLK):
                hp = hn_ps.tile([128, E], F32)
                nc.tensor.matmul(hp[:], xT[:, g * 128:(g + 1) * 128], dw_sb[:],
                                 start=True, stop=False)
                nc.tensor.matmul(hp[:], ones1[:], brow[:],
                                 start=False, stop=True)
                nc.scalar.copy(out=h_sb[:, g, :], in_=hp[:])

            # ---- 4+5. main einsum + residual + LN per block ----
            for g in range(NBLK):
                acc = blk.tile([128, E], F32)
                nc.vector.tensor_copy(out=acc[:], in_=h_sb[:, g, :])  # residual
                sT_blk = sTmm[:, g * 128:(g + 1) * 128]
                for j in range(NCHUNK):
                    yp = y_ps.tile([128, 512], F32)
                    nc.tensor.matmul(yp[:], sT_blk,
                                     w2_sb[:, j * 512:(j + 1) * 512],
                                     start=True, stop=True)
                    for k in range(4):
                        a = 4 * j + k
                        nc.vector.scalar_tensor_tensor(
                            acc[:], yp[:, k * 128:(k + 1) * 128],
                            h_sb[:, g, a:a + 1], acc[:],
                            mybir.AluOpType.mult, mybir.AluOpType.add)

                stats = small.tile([128, 6], F32)
                nc.vector.bn_stats(out=stats[:], in_=acc[:])
                mv = small.tile([128, 2], F32)
                nc.vector.bn_aggr(out=mv[:], in_=stats[:])
                std = small.tile([128, 1], F32)
                nc.scalar.activation(out=std[:], in_=mv[:, 1:2],
                                     func=mybir.ActivationFunctionType.Sqrt,
                                     bias=eps_t[:], scale=1.0)
                rstd = small.tile([128, 1], F32)
                nc.vector.reciprocal(out=rstd[:], in_=std[:])
                nrm = blk.tile([128, E], F32)
                nc.vector.tensor_scalar(
                    nrm[:], acc[:], mv[:, 0:1], rstd[:],
                    mybir.AluOpType.subtract, mybir.AluOpType.mult)
                nc.vector.tensor_mul(nrm[:], nrm[:], gam_sb[:])
                nc.vector.tensor_add(nrm[:], nrm[:], bet_sb[:])
                nc.sync.dma_start(out=out_d[g * 128:(g + 1) * 128, :], in_=nrm[:])

    nc.compile()
    return nc


def _get_nc():
    if "nc" not in _CACHE:
        _CACHE["nc"] = _build_nc()
    return _CACHE["nc"]


def kernel(x, dense_w, dense_b, concept_map, ln_gamma, ln_beta):
    global LAST_RESULT
    x = np.asarray(x, dtype=np.float32)
    dense_w = np.ascontiguousarray(np.asarray(dense_w, dtype=np.float32))
    b = np.asarray(dense_b, dtype=np.float32)
    w2 = np.ascontiguousarray(
        np.transpose(np.asarray(concept_map, dtype=np.float32), (2, 1, 0))
    ).reshape(E, E * E).astype(NPBF16)
    gam = np.ascontiguousarray(
        np.broadcast_to(np.asarray(ln_gamma, np.float32), (128, E)))
    bet = np.ascontiguousarray(
        np.broadcast_to(np.asarray(ln_beta, np.float32), (128, E)))

    nc = _get_nc()
    shared = {"dense_w": dense_w.astype(NPBF16), "b_col": b.reshape(E, 1),
              "b_row": b.reshape(1, E).astype(NPBF16), "w2": w2,
              "gamma_rep": gam, "beta_rep": bet}
    in_maps = [dict(shared, xT=np.ascontiguousarray(x[bi].T).astype(NPBF16))
               for bi in range(B)]
    res = run_bass_kernel_spmd(nc, in_maps, core_ids=list(range(B)))
    LAST_RESULT = res
    out = np.stack([res.results[bi]["out"] for bi in range(B)]).astype(np.float32)
    return out


if __name__ == "__main__":
    rng = np.random.default_rng(0)
    inputs = {
        "x": rng.standard_normal((B, S, E)).astype(np.float32),
        "dense_w": rng.standard_normal((E, E)).astype(np.float32) * 0.02,
        "dense_b": np.zeros(E, np.float32),
        "concept_map": rng.standard_normal((E, E, E)).astype(np.float32) * 0.02,
        "ln_gamma": np.ones(E, np.float32),
        "ln_beta": np.zeros(E, np.float32),
    }
    out = kernel(**inputs)
    print("out", out.shape, out.dtype, float(np.abs(out).max()))



# revision 6
# speedup vs baseline: 1.7020x; 1.7020x over previous
"""Trainium2 Bass kernel for nn_ConceptLayer (B=8, S=2048, E=128).

out[b,s,c] = LN( einsum('sa,sp,cap->sc', h[b], s_seq[b], W) + h[b] )
  h = x @ dense_w + dense_b
  s_seq = decayed prefix sum of h along s (s_i = (s_{i-1}+h_{i-1})/1.2)

Sharding: data-parallel over batch, one sample per NeuronCore (8 cores).
concept_map replicated (host-pretransposed to [p, (a,c)] bf16); x is
host-pretransposed to xT [e, t] so no on-device transposes are needed.

Per-core pipeline:
  1. hT-chunk matmuls (dense_w.T @ xT, fp32); PSUM->SBUF copy applies
     (+bias)*1/DECAY producing the scan feed hTp (shifted one col)
  2. s_seq^T via tensor_tensor_scan (state = state/d + (h+b)/d) in fp32,
     downcast to bf16 (sTmm) for the tensor-engine operand
  3. h natural layout per 128-token block: matmul(lhsT=xT slice, rhs=dense_w)
     plus a K=1 ones-x-bias matmul into the same PSUM
  4. per block: 32 matmuls N=512 (lhsT = sT block stationary, rhs = W2
     slices) -> Y chunks in PSUM.  The contraction acc[t,c] += Y[t,(a,c)]
     * h[t,a] is split across three engines with separate accumulators
     (merged at the end): DVE scalar_tensor_tensor direct from PSUM,
     GpSimd scalar_tensor_tensor direct from PSUM, and ACT scale-copies
     (Y*h -> fp16 scratch) folded by cheap fp16 2x-mode DVE adds.
  5. LayerNorm: stats on DVE (bn_stats/bn_aggr), normalize applied on the
     ACT engine (Identity with per-partition scale/bias), gamma/beta on
     DVE, DMA out
"""

import os
import sys

import numpy as np

for _p in ("/opt/trn_rl_repo",):
    if _p not in sys.path and os.path.isdir(_p):
        sys.path.insert(0, _p)

import concourse.bass as bass
import concourse.bacc as bacc
import concourse.tile as tile
from concourse import mybir
from concourse.bass_utils import run_bass_kernel_spmd

B, S, E = 8, 2048, 128
DECAY = 1.2
LN_EPS = 1e-3
NBLK = S // 128          # 16 token blocks per core
NCHUNK = (E * E) // 512  # 32 matmul chunks of 512 (a,c) columns per block
F32 = mybir.dt.float32
BF16 = mybir.dt.bfloat16
F16 = mybir.dt.float16
NPBF16 = mybir.dt.np(BF16)

# Which engine(s) consume chunk j of each block (GPSIMD can neither touch
# PSUM nor run scalar_tensor_tensor, so Pool-bound chunks are scaled into
# fp16 SBUF scratch by ACT first and Pool only does tensor_tensor adds):
#   V = DVE scalar_tensor_tensor (direct from PSUM, fp32 acc)
#   A = ACT scale-copy (Y*h -> fp16 scratch) + DVE fp16 tree-add
#   B = ACT scale-copy (Y*h -> fp16 scratch) + Pool fp16 tree-add
SCHED = list("AVBAVBAVBVAVBAVBAVBVAVBAVBAVBVAB")
assert len(SCHED) == NCHUNK and len(SCHED) == 32
assert SCHED.count("V") == 12 and SCHED.count("A") == 10 and SCHED.count("B") == 10

_CACHE = {}
LAST_RESULT = None  # BassKernelResults of the most recent run (for test.py)


def _build_nc():
    nc = bacc.Bacc(None, target_bir_lowering=False)

    xT_d = nc.declare_dram_parameter("xT", [E, S], BF16, isOutput=False)
    dw_d = nc.declare_dram_parameter("dense_w", [E, E], BF16, isOutput=False)
    bc_d = nc.declare_dram_parameter("b_col", [E, 1], F32, isOutput=False)
    br_d = nc.declare_dram_parameter("b_row", [1, E], BF16, isOutput=False)
    w2_d = nc.declare_dram_parameter("w2", [E, E * E], BF16, isOutput=False)
    gam_d = nc.declare_dram_parameter("gamma_rep", [128, E], F32, isOutput=False)
    bet_d = nc.declare_dram_parameter("beta_rep", [128, E], F32, isOutput=False)
    out_d = nc.declare_dram_parameter("out", [S, E], F32, isOutput=True)

    NW2 = 4  # load w2 in 4 slices so early matmuls don't wait on 4MB
    W2COLS = (E * E) // NW2
    CPW = W2COLS // 512  # chunks per w2 slice

    with tile.TileContext(nc) as tc:
        with (
            tc.tile_pool(name="singles", bufs=1) as singles,
            tc.tile_pool(name="blk", bufs=4) as blk,
            tc.tile_pool(name="small", bufs=4) as small,
            tc.tile_pool(name="atmp", bufs=6) as atmp,
        ):
            # ---- resident tensors ----
            xT = singles.tile([E, S], BF16)
            nc.sync.dma_start(out=xT[:], in_=xT_d[:])
            dw_sb = singles.tile([E, E], BF16)
            nc.sync.dma_start(out=dw_sb[:], in_=dw_d[:])
            bcol = singles.tile([E, 1], F32)
            nc.sync.dma_start(out=bcol[:], in_=bc_d[:])
            brow = singles.tile([1, E], BF16)
            nc.sync.dma_start(out=brow[:], in_=br_d[:])
            w2_sb = []
            for q in range(NW2):
                w2q = singles.tile([E, W2COLS], BF16, name=f"w2_{q}")
                nc.sync.dma_start(
                    out=w2q[:], in_=w2_d[:, q * W2COLS:(q + 1) * W2COLS])
                w2_sb.append(w2q)
            gam_sb = singles.tile([128, E], F32)
            nc.sync.dma_start(out=gam_sb[:], in_=gam_d[:])
            bet_sb = singles.tile([128, E], F32)
            nc.sync.dma_start(out=bet_sb[:], in_=bet_d[:])

            eps_t = singles.tile([128, 1], F32)
            nc.vector.memset(eps_t[:], LN_EPS)
            dinv = singles.tile([128, 512], F32)
            nc.vector.memset(dinv[:], 1.0 / DECAY)
            ones1 = singles.tile([1, 128], BF16)
            nc.vector.memset(ones1[:], 1.0)

            hTp = singles.tile([E, S + 1], F32)   # col j+1 = (h_j+b)/d, col0 = 0
            sT32 = singles.tile([E, S], F32)
            sTmm = singles.tile([E, S], BF16)
            h_sb = singles.tile([128, NBLK, E], F32)  # h natural, blocked

            nc.vector.memset(hTp[:, 0:1], 0.0)

            with (
                tc.tile_pool(name="h_ps", bufs=2, space="PSUM") as h_ps,
                tc.tile_pool(name="hn_ps", bufs=2, space="PSUM") as hn_ps,
            ):
                # ---- 1. hT chunks = dense_w.T @ xT; scan feed with (+b)/d
                for q in range(4):
                    hp = h_ps.tile([E, 512], F32)
                    nc.tensor.matmul(hp[:], dw_sb[:],
                                     xT[:, q * 512:(q + 1) * 512],
                                     start=True, stop=True)
                    nc.vector.tensor_scalar(
                        hTp[:, q * 512 + 1:(q + 1) * 512 + 1], hp[:], bcol[:],
                        1.0 / DECAY, mybir.AluOpType.add, mybir.AluOpType.mult)

                # ---- 2. decay prefix scan -> sT (scan DVE, cast ACT) ----
                for q in range(4):
                    lo, hi = q * 512, (q + 1) * 512
                    init = 0.0 if q == 0 else sT32[:, lo - 1:lo]
                    nc.vector.tensor_tensor_scan(
                        sT32[:, lo:hi], dinv[:], hTp[:, lo:hi], init,
                        mybir.AluOpType.mult, mybir.AluOpType.add)
                    nc.scalar.copy(out=sTmm[:, lo:hi], in_=sT32[:, lo:hi])

                # ---- 3. h natural: h_blk = xT_blk.T @ dense_w + 1s*b ----
                for g in range(NBLK):
                    hp = hn_ps.tile([128, E], F32)
                    nc.tensor.matmul(hp[:], xT[:, g * 128:(g + 1) * 128],
                                     dw_sb[:], start=True, stop=False)
                    nc.tensor.matmul(hp[:], ones1[:], brow[:],
                                     start=False, stop=True)
                    nc.scalar.copy(out=h_sb[:, g, :], in_=hp[:])

            # ---- 4+5. main einsum + residual + LN per block ----
            with tc.tile_pool(name="y_ps", bufs=8, space="PSUM") as y_ps:
                for g in range(NBLK):
                    acc_v = blk.tile([128, E], F32, name="acc_v")
                    nc.scalar.copy(out=acc_v[:], in_=h_sb[:, g, :])  # residual
                    acc_p = blk.tile([128, E], F32, name="acc_p")
                    nc.gpsimd.memset(acc_p[:], 0.0)
                    acc_a = blk.tile([128, E], F32, name="acc_a")
                    nc.vector.memset(acc_a[:], 0.0)

                    sT_blk = sTmm[:, g * 128:(g + 1) * 128]
                    for j in range(NCHUNK):
                        yp = y_ps.tile([128, 512], F32)
                        w2q = w2_sb[j // CPW]
                        jj = j % CPW
                        nc.tensor.matmul(yp[:], sT_blk,
                                         w2q[:, jj * 512:(jj + 1) * 512],
                                         start=True, stop=True)
                        cls = SCHED[j]
                        if cls == "V":
                            for k in range(4):
                                a = 4 * j + k
                                nc.vector.scalar_tensor_tensor(
                                    acc_v[:], yp[:, k * 128:(k + 1) * 128],
                                    h_sb[:, g, a:a + 1], acc_v[:],
                                    mybir.AluOpType.mult, mybir.AluOpType.add)
                        else:  # 'A' or 'B'
                            tmp = atmp.tile([128, 4, 128], F16, name="tmp")
                            for k in range(4):
                                a = 4 * j + k
                                nc.scalar.activation(
                                    out=tmp[:, k, :],
                                    in_=yp[:, k * 128:(k + 1) * 128],
                                    func=mybir.ActivationFunctionType.Copy,
                                    scale=h_sb[:, g, a:a + 1])
                            eng = nc.vector if cls == "A" else nc.gpsimd
                            acc = acc_a if cls == "A" else acc_p
                            u = atmp.tile([128, 2, 128], F16, name="u")
                            eng.tensor_tensor(
                                u[:], tmp[:, 0:2, :], tmp[:, 2:4, :],
                                mybir.AluOpType.add)
                            v = atmp.tile([128, 128], F16, name="v")
                            eng.tensor_tensor(
                                v[:], u[:, 0, :], u[:, 1, :],
                                mybir.AluOpType.add)
                            eng.tensor_tensor(
                                acc[:], acc[:], v[:],
                                mybir.AluOpType.add)

                    nc.vector.tensor_add(acc_v[:], acc_v[:], acc_p[:])
                    nc.vector.tensor_add(acc_v[:], acc_v[:], acc_a[:])

                    stats = small.tile([128, 6], F32)
                    nc.vector.bn_stats(out=stats[:], in_=acc_v[:])
                    mv = small.tile([128, 2], F32)
                    nc.vector.bn_aggr(out=mv[:], in_=stats[:])
                    std = small.tile([128, 1], F32)
                    nc.scalar.activation(out=std[:], in_=mv[:, 1:2],
                                         func=mybir.ActivationFunctionType.Sqrt,
                                         bias=eps_t[:], scale=1.0)
                    rstd = small.tile([128, 1], F32)
                    nc.vector.reciprocal(out=rstd[:], in_=std[:])
                    # negmr = -mean * rstd
                    negmr = small.tile([128, 1], F32)
                    nc.vector.tensor_scalar(
                        negmr[:], mv[:, 0:1], rstd[:], -1.0,
                        mybir.AluOpType.mult, mybir.AluOpType.mult)
                    nrm = blk.tile([128, E], F32, name="nrm")
                    nc.scalar.activation(
                        out=nrm[:], in_=acc_v[:],
                        func=mybir.ActivationFunctionType.Identity,
                        bias=negmr[:], scale=rstd[:])
                    nc.vector.tensor_mul(nrm[:], nrm[:], gam_sb[:])
                    nc.vector.tensor_add(nrm[:], nrm[:], bet_sb[:])
                    nc.sync.dma_start(out=out_d[g * 128:(g + 1) * 128, :],
                                      in_=nrm[:])

    nc.compile()
    return nc


def _get_nc():
    if "nc" not in _CACHE:
        _CACHE["nc"] = _build_nc()
    return _CACHE["nc"]


def kernel(x, dense_w, dense_b, concept_map, ln_gamma, ln_beta):
    global LAST_RESULT
    x = np.asarray(x, dtype=np.float32)
    dense_w = np.ascontiguousarray(np.asarray(dense_w, dtype=np.float32))
    b = np.asarray(dense_b, dtype=np.float32)
    w2 = np.ascontiguousarray(
        np.transpose(np.asarray(concept_map, dtype=np.float32), (2, 1, 0))
    ).reshape(E, E * E).astype(NPBF16)
    gam = np.ascontiguousarray(
        np.broadcast_to(np.asarray(ln_gamma, np.float32), (128, E)))
    bet = np.ascontiguousarray(
        np.broadcast_to(np.asarray(ln_beta, np.float32), (128, E)))

    nc = _get_nc()
    shared = {"dense_w": dense_w.astype(NPBF16), "b_col": b.reshape(E, 1),
              "b_row": b.reshape(1, E).astype(NPBF16), "w2": w2,
              "gamma_rep": gam, "beta_rep": bet}
    in_maps = [dict(shared, xT=np.ascontiguousarray(x[bi].T).astype(NPBF16))
               for bi in range(B)]
    res = run_bass_kernel_spmd(nc, in_maps, core_ids=list(range(B)))
    LAST_RESULT = res
    out = np.stack([res.results[bi]["out"] for bi in range(B)]).astype(np.float32)
    return out


if __name__ == "__main__":
    rng = np.random.default_rng(0)
    inputs = {
        "x": rng.standard_normal((B, S, E)).astype(np.float32),
        "dense_w": rng.standard_normal((E, E)).astype(np.float32) * 0.02,
        "dense_b": np.zeros(E, np.float32),
        "concept_map": rng.standard_normal((E, E, E)).astype(np.float32) * 0.02,
        "ln_gamma": np.ones(E, np.float32),
        "ln_beta": np.zeros(E, np.float32),
    }
    out = kernel(**inputs)
    print("out", out.shape, out.dtype, float(np.abs(out).max()))
